# revision 37
# baseline (speedup 1.0000x reference)
"""MoE layer (top-2 of 8 experts, H=1024, FFN=4096) on 8 TRN2 NeuronCores.

Expert-parallel: core e holds expert e's weights. The (tiny) router runs on
host; tokens are gathered per-expert into capacity-padded batches, each core
runs the expert FFN (x @ w1.T -> +b1 -> gelu -> @ w2.T -> +b2 -> *gate) and
the host scatter-adds the two expert contributions per token back together.

Device layout per core (C = per-expert token capacity, multiple of 64):
  GEMM1  h[f, c] = w1t[h, f].T @ xT[h, c]   (F on partitions, tokens free)
  GEMM2  y[c, n] = h[f, c].T  @ w2t[f, n]   (tokens on partitions, H free)
b1 is per-partition in GEMM1's output (activation bias), b2 is broadcast
along partitions via a ones-row matmul trick and folded into the first
slab's PSUM eviction, gate is applied per-partition on ScalarE at the end.

GEMMs run in bf16 (fp32 matmul on this PE is 2 passes at half clock = 4x
slower; fp16 hangs the exec unit). PSUM accumulation, biases, gelu and the
final combine stay fp32; end-to-end absmax-relative error vs the fp32
reference is ~3.5e-3.
"""

import os

os.environ.setdefault("NEURON_RT_RESET_CORES", "1")

import ml_dtypes
import numpy as np

import concourse.bass as bass  # noqa: F401  (bass types via bacc)
import concourse.mybir as mybir
from concourse import bacc
from concourse.tile import TileContext
from concourse.bass_utils import run_bass_kernel_spmd

H = 1024
E = 8
F = 4096
TOPK = 2
P = 128
N_CORES = 8
FP32 = mybir.dt.float32
BF16 = mybir.dt.bfloat16

_cache: dict = {}

# Test-harness knobs (harness-safe defaults): set TRACE=True before calling
# kernel() to profile the device run; exec time lands in LAST_EXEC_TIME_NS.
TRACE = False
LAST_EXEC_TIME_NS = None


def _build(C: int):
    """Build + compile the per-core expert-FFN program for capacity C.

    C is a multiple of 64. Token tiles are 128 wide except a possible
    final partial tile (the PE handles M<128 with partial-partition psum).
    """
    assert C % 64 == 0
    n_ct = -(-C // P)          # token tiles (last one may be partial)
    # Near-even c-block widths <=512. A narrow tail block would make its
    # GEMM1 matmuls LDWEIGHTS-bound; even widths keep every matmul's weight
    # load hidden under its streaming time. Block starts must stay
    # 128-aligned (the GEMM2 token-tile index is coff//128), so all widths
    # are multiples of 128 except the last, which may carry a +64 tail.
    nb = -(-C // 512)
    u, rem = divmod(C, P)             # full 128-token units + 0/64 tail
    units = [u // nb + (1 if i < u % nb else 0) for i in range(nb)]
    widths = [un * P for un in units]
    widths[-1] += rem
    assert all(0 < w <= 512 for w in widths) and sum(widths) == C
    assert all(w % P == 0 for w in widths[:-1])
    cbs = []
    off = 0
    for w in widths:
        cbs.append((off, w))
        off += w

    NTH = 8            # number of F slabs ("eighths")
    FT = F // NTH      # 512 F columns per slab
    MF = FT // P       # 4 m-tiles of 128 per slab
    KH = H // P        # 8 contraction tiles for GEMM1

    nc = bacc.Bacc("TRN2", target_bir_lowering=False, debug=False,
                   num_devices=N_CORES)

    xT = nc.dram_tensor("xT", [H, C], BF16, kind="ExternalInput")
    w1t = nc.dram_tensor("w1t", [H, F], BF16, kind="ExternalInput")
    w2t = nc.dram_tensor("w2t", [F, H], BF16, kind="ExternalInput")
    b1c = nc.dram_tensor("b1c", [P, F // P], FP32, kind="ExternalInput")
    b2r = nc.dram_tensor("b2r", [1, H], FP32, kind="ExternalInput")
    gate = nc.dram_tensor("gate", [P, n_ct], FP32, kind="ExternalInput")
    out = nc.dram_tensor("out", [C, H], FP32, kind="ExternalOutput")

    xT_v = xT.rearrange("(k p) c -> p k c", p=P)     # [128, 8, C]
    w1_v = w1t.rearrange("(k p) f -> p k f", p=P)    # [128, 8, F]
    w2_v = w2t.rearrange("(f p) h -> p f h", p=P)    # [128, 32, H]

    GELU = mybir.ActivationFunctionType.Gelu
    ADD = mybir.AluOpType.add

    with TileContext(nc) as tc:
        with (
            tc.tile_pool(name="const", bufs=1) as constp,
            tc.tile_pool(name="xp", bufs=1) as xp,
            tc.tile_pool(name="w1p", bufs=2) as w1p,
            tc.tile_pool(name="w2p", bufs=2) as w2p,
            tc.tile_pool(name="hp", bufs=4) as hp,
            tc.tile_pool(name="yp", bufs=1) as yp,
            tc.tile_pool(name="op", bufs=2) as op,
            tc.tile_pool(name="ps", bufs=4, space="PSUM") as ps,
        ):
            # DMA emission order = arrival order. Smallest first (the first PE
            # instruction, the b2 broadcast matmul, waits on b2row), then the
            # first GEMM1 group's inputs (x of c-block 0 + th0 weights), then
            # the rest of x.
            b2row = constp.tile([P, H], FP32, tag="b2row")
            nc.vector.memset(b2row[:], 0.0)
            nc.sync.dma_start(out=b2row[0:1, :], in_=b2r[:])
            b1_sb = constp.tile([P, F // P], FP32, tag="b1")
            nc.sync.dma_start(out=b1_sb[:], in_=b1c[:])
            gate_sb = constp.tile([P, n_ct], FP32, tag="gate")
            nc.sync.dma_start(out=gate_sb[:], in_=gate[:])

            x_sb = {}

            def load_x(cbi):
                coff, cw = cbs[cbi]
                t = xp.tile([P, KH, cw], BF16, tag=f"x_{cbi}", name=f"x_{cbi}")
                nc.sync.dma_start(out=t[:], in_=xT_v[:, :, coff:coff + cw])
                x_sb[cbi] = t

            load_x(0)

            y_sb = [yp.tile([P, H], FP32, tag=f"y{j}", name=f"y{j}") for j in range(n_ct)]

            # b2 broadcast across partitions: psum = onesrow.T @ b2row.
            # Emitted before the GEMM stream: PE is still waiting on its
            # first weight tiles here, so these two matmuls are free.
            ones_t = constp.tile([P, P], FP32, tag="ones")
            nc.vector.memset(ones_t[:], 0.0)
            nc.vector.memset(ones_t[0:1, :], 1.0)
            b2bc = constp.tile([P, H], FP32, tag="b2bc")
            for n2 in range(2):
                pt = ps.tile([P, 512], FP32, tag="ps2", name=f"psb2_{n2}")
                nc.tensor.matmul(pt[:], ones_t[:], b2row[:, n2 * 512:(n2 + 1) * 512],
                                 start=True, stop=True)
                nc.scalar.copy(b2bc[:, n2 * 512:(n2 + 1) * 512], pt[:])

            for th in range(NTH):
                w1_t = w1p.tile([P, KH, FT], BF16, tag="w1", name=f"w1_{th}")
                nc.sync.dma_start(out=w1_t[:], in_=w1_v[:, :, th * FT:(th + 1) * FT])
                w2_t = w2p.tile([P, MF, H], BF16, tag="w2", name=f"w2_{th}")
                nc.sync.dma_start(out=w2_t[:],
                                  in_=w2_v[:, th * MF:(th + 1) * MF, :])
                if th == 0:
                    for cbi in range(1, len(cbs)):
                        load_x(cbi)

                for cbi, (coff, cw) in enumerate(cbs):
                    h_t = hp.tile([P, MF, cw], BF16, tag="h")
                    for m in range(MF):
                        pt = ps.tile([P, cw], FP32, tag="ps1")
                        for k in range(KH):
                            nc.tensor.matmul(
                                pt[:],
                                w1_t[:, k, m * P:(m + 1) * P],
                                x_sb[cbi][:, k, :],
                                start=(k == 0), stop=(k == KH - 1),
                            )
                        nc.scalar.activation(
                            h_t[:, m, :], pt[:], GELU,
                            bias=b1_sb[:, th * MF + m:th * MF + m + 1],
                        )
                    for ct in range(-(-cw // P)):
                        j = (coff // P) + ct
                        ctw = min(P, cw - ct * P)
                        for n2 in range(2):
                            pt2 = ps.tile([P, 512], FP32, tag="ps2")
                            for m in range(MF):
                                nc.tensor.matmul(
                                    pt2[:ctw, :],
                                    h_t[:, m, ct * P:ct * P + ctw],
                                    w2_t[:, m, n2 * 512:(n2 + 1) * 512],
                                    start=(m == 0), stop=(m == MF - 1),
                                )
                            ys = y_sb[j][:ctw, n2 * 512:(n2 + 1) * 512]
                            if th == 0:
                                # fold the b2 add in here: y ends up as
                                # sum(psums) + b2, so the final pass is a
                                # single gate multiply
                                nc.vector.tensor_tensor(
                                    ys, pt2[:ctw, :],
                                    b2bc[:ctw, n2 * 512:(n2 + 1) * 512], ADD)
                            else:
                                nc.vector.tensor_tensor(ys, ys, pt2[:ctw, :], ADD)

            COPY = mybir.ActivationFunctionType.Copy
            for j in range(n_ct):
                jw = min(P, C - j * P)
                o_t = op.tile([P, H], FP32, tag="o")
                # gate multiply on ScalarE (idle at the tail; DVE is busy
                # with the last y accumulations)
                nc.scalar.activation(o_t[:jw, :], y_sb[j][:jw, :], COPY,
                                     scale=gate_sb[:jw, j:j + 1])
                nc.sync.dma_start(out=out[j * P:j * P + jw, :], in_=o_t[:jw, :])

    nc.compile()
    return nc


def _route(x: np.ndarray, router_w: np.ndarray):
    """Host router: top-2 expert ids + softmax gates per token."""
    logits = x @ router_w.T                                   # [T, E]
    top_i = np.argsort(-logits, axis=1, kind="stable")[:, :TOPK]
    top_v = np.take_along_axis(logits, top_i, axis=1)
    mx = top_v.max(axis=1, keepdims=True)
    ex = np.exp(top_v - mx)
    rw = ex / ex.sum(axis=1, keepdims=True)
    return top_i, rw.astype(np.float32)


def kernel(hidden_states, router_w, w1, b1, w2, b2):
    hidden_states = np.ascontiguousarray(np.asarray(hidden_states, np.float32))
    router_w = np.ascontiguousarray(np.asarray(router_w, np.float32))
    w1 = np.asarray(w1, np.float32)
    b1 = np.asarray(b1, np.float32)
    w2 = np.asarray(w2, np.float32)
    b2 = np.asarray(b2, np.float32)

    B, S, _ = hidden_states.shape
    T = B * S
    x = hidden_states.reshape(T, H)

    top_i, rw = _route(x, router_w)

    sel_idx = []
    sel_gate = []
    for e in range(E):
        mask = top_i == e                                     # [T, K]
        rows = np.nonzero(mask.any(axis=1))[0]
        g = rw[rows[:, None], np.argmax(mask[rows], axis=1)[:, None]][:, 0]
        sel_idx.append(rows)
        sel_gate.append(g.astype(np.float32))

    # One job per (expert, token-chunk). Normally each expert fits in one
    # chunk and a single 8-core SPMD round runs everything; with an extreme
    # routing skew an expert's batch is split into <=C_MAX chunks (bounded
    # by SBUF) and extra rounds run.
    C_MAX = 2048
    jobs = []                                   # (expert, rows, gates)
    for e in range(E):
        rows, g = sel_idx[e], sel_gate[e]
        for off in range(0, max(len(rows), 1), C_MAX):
            jobs.append((e, rows[off:off + C_MAX], g[off:off + C_MAX]))

    n_rounds = -(-len(jobs) // N_CORES)
    cmax = max(len(r) for _, r, _ in jobs)
    C = max(P, -(-cmax // 64) * 64)

    if C not in _cache:
        _cache[C] = _build(C)
    nc = _cache[C]

    w_bf = {}
    def expert_inputs(e):
        if e not in w_bf:
            w_bf[e] = {
                "w1t": np.ascontiguousarray(w1[e].T).astype(ml_dtypes.bfloat16),
                "w2t": np.ascontiguousarray(w2[e].T).astype(ml_dtypes.bfloat16),
                "b1c": np.ascontiguousarray(b1[e].reshape(F // P, P).T),
                "b2r": np.ascontiguousarray(b2[e].reshape(1, H)),
            }
        return w_bf[e]

    global LAST_EXEC_TIME_NS
    LAST_EXEC_TIME_NS = 0
    out = np.zeros((T, H), np.float32)
    n_ct = -(-C // P)
    for r in range(n_rounds):
        batch = jobs[r * N_CORES:(r + 1) * N_CORES]
        while len(batch) < N_CORES:
            batch.append((0, sel_idx[0][:0], sel_gate[0][:0]))
        in_maps = []
        for e, rows, g in batch:
            n_e = len(rows)
            xT_e = np.zeros((H, C), ml_dtypes.bfloat16)
            xT_e[:, :n_e] = x[rows].T.astype(ml_dtypes.bfloat16)
            gate_e = np.zeros(n_ct * P, np.float32)
            gate_e[:n_e] = g
            in_maps.append({
                "xT": xT_e,
                "gate": np.ascontiguousarray(gate_e.reshape(n_ct, P).T),
                **expert_inputs(e),
            })

        res = run_bass_kernel_spmd(nc, in_maps, list(range(N_CORES)), trace=TRACE)
        if res.exec_time_ns:
            LAST_EXEC_TIME_NS += res.exec_time_ns

        for core, (e, rows, g) in enumerate(batch):
            if len(rows):
                # row indices are unique within one job, so += is safe
                out[rows] += res.results[core]["out"][:len(rows)]

    return out.reshape(B, S, H)
